# revision 41
# baseline (speedup 1.0000x reference)
"""LocalWindowAttention Trainium2 Bass kernel (v4: uniform-K fp16).

Full-input contract: kernel(**inputs) takes the unsharded tensors
(x:[8,192,224,224], Wq/Wk/Wv/Wo:[192,192], bq/bk/bv/bo:[192]) and
returns the full [8,192,224,224] output.  Data-parallel over batch
across 8 NeuronCores (1 image per core), weights replicated.

Math notes (vs reference):
  - H=W=224 divide by ws=7, so the reference's reflect-pad is a no-op.
  - V-bias folded out: softmax rows sum to 1, so bo_eff = Wo@bv + bo.
  - no max-subtraction in softmax: scores/sqrt(C) are O(+-6); exp fits
    fp16 range and fp32 PSUM accumulates the sums.
  - All matmuls fp16 (1 cyc/row), fp32 PSUM accumulation.

Performance-critical structure (measured on this part):
  - Back-to-back matmuls pipeline at ~N cycles ONLY when the tile
    config (rounded K x M) stays constant; alternating K=128/K=64
    costs ~2.7x.  So ALL channel-contraction matmuls are padded to
    K=128: the 64-row second channel-chunk of x/q/k and the weight
    chunks are zero-padded, and the q/k convs emit M=128 with 64 zero
    weight columns so the padded q1/k1 rows are zero by construction.
  - S^T formulation (lhsT=k, rhs=q) avoids P transposes entirely;
    window pairs use tile_position col groups 0/64 (PSUM partitions
    0-48 / 64-112, physically disjoint per-partition SRAMs).
  - V is computed pixel-major via a transposed conv (stationary =
    x-window chunk); PV uses V as stationary (moving = P^T),
    giving channel-major O^T.  Even/odd PV outputs overlap in
    partitions and therefore go to SEPARATE banks (concurrent
    row-tiled drains into one bank are a fatal HW collision).
  - Softmax: column sums via M=64 ones-stationary matmuls (sums
    replicated across partitions), fast approximate reciprocal,
    gpsimd multiply -> P^T fp16.
"""

import math
from contextlib import ExitStack

import numpy as np

import concourse.bacc as bacc
import concourse.bass as bass
import concourse.tile as tile
from concourse import mybir
from concourse.ap import AP
from concourse.bass_utils import run_bass_kernel_spmd

F32 = mybir.dt.float32
F16 = mybir.dt.float16

B, C, H, W = 8, 192, 224, 224
WS = 7
NSTRIP = H // WS            # 32 strips (one window-row each)
SP = WS * W                 # 1568 pixels per strip
NW = W // WS                # 32 windows per strip
WP = WS * WS                # 49 pixels per window
NT = 392                    # N-tile = 8 windows
NGRP = SP // NT             # 4 groups per strip
C0, C1 = 128, 64            # channel chunks (192 = 128 + 64)
SCALE = 1.0 / math.sqrt(C)
GP = 8                      # window pairs per attention group

_CACHE = {}

# debug: 1=convs only (final conv reads q), 2=+V, 3=+QK/softmax, 4=full
STAGE = 4


def _build():
    nc = bacc.Bacc(None, target_bir_lowering=False)

    x_d = nc.dram_tensor("x", [C, H * W], F16, kind="ExternalInput")
    y_d = nc.dram_tensor("y", [C, H, W], F32, kind="ExternalOutput")
    w_d = {
        n: nc.dram_tensor(n, [C, C], F16, kind="ExternalInput")
        for n in ("wqT", "wkT", "wvT", "woT")
    }
    b_d = {
        n: nc.dram_tensor(n, [C, 1], F32, kind="ExternalInput")
        for n in ("bq", "bk", "bo")
    }
    ones_d = nc.inline_tensor(np.ones((C0, 64), dtype=np.float16), name="ones64")

    with tile.TileContext(nc) as tc, ExitStack() as ctx:
        const = ctx.enter_context(tc.tile_pool(name="const", bufs=1))

        # weights as two K=128 chunks of [128, 256]:
        #   wt[n][0]: rows = in-ch 0-127;  wt[n][1]: rows 0-63 = in-ch
        #   128-191, rows 64-127 = 0.  cols 0-191 = out-ch, 192-255 = 0
        #   (so the M=128 "m1" stationary wt[:,128:256] has 64 zero cols
        #   and the conv's second output chunk lands zero-padded).
        wt = {}
        for n in ("wqT", "wkT", "wvT", "woT"):
            t0 = const.tile([C0, 256], F16, tag=f"{n}0")
            t1 = const.tile([C0, 256], F16, tag=f"{n}1")
            nc.vector.memset(t0[:], 0.0)
            nc.vector.memset(t1[:], 0.0)
            nc.sync.dma_start(t0[:, 0:C], w_d[n][0:C0, :])
            nc.sync.dma_start(t1[0:C1, 0:C], w_d[n][C0:C, :])
            wt[n] = (t0, t1)
        bias = {}
        for n in ("bq", "bk", "bo"):
            t0 = const.tile([C0, 1], F32, tag=f"{n}0")
            t1 = const.tile([C0, 1], F32, tag=f"{n}1")
            nc.vector.memset(t1[:], 0.0)
            nc.sync.dma_start(t0[:], b_d[n][0:C0, :])
            nc.sync.dma_start(t1[0:C1], b_d[n][C0:C, :])
            bias[n] = (t0, t1)
        ones = const.tile([C0, 64], F16, tag="ones")
        nc.sync.dma_start(ones[:], ones_d[:, :])

        xp = ctx.enter_context(tc.tile_pool(name="xp", bufs=3))
        qkp = ctx.enter_context(tc.tile_pool(name="qkp", bufs=3))
        vtp = ctx.enter_context(tc.tile_pool(name="vtp", bufs=3))
        smp = ctx.enter_context(tc.tile_pool(name="smp", bufs=2))
        otp = ctx.enter_context(tc.tile_pool(name="otp", bufs=3))
        outp = ctx.enter_context(tc.tile_pool(name="outp", bufs=3))

        # PSUM pools: 8 banks.  Matmul outputs that overlap in partition
        # range must be in different banks (concurrent drains collide).
        psb = ctx.enter_context(
            tc.tile_pool(name="psb", bufs=3, space=bass.MemorySpace.PSUM)
        )  # [128,392] conv outs (m0+m1) and PV m0 even/odd
        pso = ctx.enter_context(
            tc.tile_pool(name="pso", bufs=2, space=bass.MemorySpace.PSUM)
        )  # colsums + PV m1 even/odd
        psv = ctx.enter_context(
            tc.tile_pool(name="psv", bufs=2, space=bass.MemorySpace.PSUM)
        )  # [128, 2, 192] V pairs (even at 0-48, odd at 64-112)
        pssc = ctx.enter_context(
            tc.tile_pool(name="pssc", bufs=1, space=bass.MemorySpace.PSUM)
        )  # [128, GP, 49] S^T

        for s in range(NSTRIP):
            # ---- x arrives from the host already window-major fp16
            # ([C, strip, w, r, cc] order): DMA straight into xwm.
            # xwm[1] rows 64-127 are zero (K=128 padding).
            # (tiles padded to 1600 cols: the xw2 build below reads 64-col
            # windows that overrun the last window by 15 cols)
            xwm = (
                xp.tile([C0, 1600], F16, tag="xw0", name="xw0t", bufs=3),
                xp.tile([C0, 1600], F16, tag="xw1", name="xw1t", bufs=3),
            )
            nc.sync.dma_start(
                xwm[0][:, 0:SP], x_d[0:C0, SP * s : SP * (s + 1)]
            )
            nc.sync.dma_start(
                xwm[1][0:C1, 0:SP], x_d[C0:C, SP * s : SP * (s + 1)]
            )
            nc.gpsimd.memset(xwm[0][:, SP:1600], 0.0)
            nc.gpsimd.memset(xwm[1][0:C1, SP:1600], 0.0)
            nc.gpsimd.memset(xwm[1][C1:C0], 0.0)
            # 64-aligned window-pair layout for the V conv: pair p occupies
            # cols 128p..128p+128, window a at +0, b at +64 (each 49 real
            # cols + 15 don't-care).  One overlapping-read copy per chunk.
            xw2 = (
                xp.tile([C0, 2048], F16, tag="xq0", name="xq0t", bufs=3),
                xp.tile([C0, 2048], F16, tag="xq1", name="xq1t", bufs=3),
            )
            for ki in range(2):
                src_ap = AP(
                    tensor=xwm[ki][:].tensor,
                    offset=xwm[ki][:].offset,
                    ap=[list(xwm[ki][:].ap[0]), [98, 16], [49, 2], [1, 64]],
                )
                nc.gpsimd.tensor_copy(
                    xw2[ki][:].rearrange(
                        "c (p w e) -> c p w e", w=2, e=64
                    ),
                    src_ap,
                )

            # ---- q, k convs -> two K=128-padded [128, 1568] chunks each
            # evacs alternate ACT/DVE per N-tile so neither engine gates
            def conv_qk(wname, bname, tag, phase):
                out0 = qkp.tile([C0, SP], F16, tag=f"{tag}0")
                out1 = qkp.tile([C0, SP], F16, tag=f"{tag}1")
                for nt in range(NGRP):
                    sl = slice(NT * nt, NT * nt + NT)
                    for mi, ot in ((0, out0), (1, out1)):
                        ps = psb.tile([C0, NT], F32, tag="big")
                        for ki in range(2):
                            nc.tensor.matmul(
                                ps[:],
                                wt[wname][ki][:, 128 * mi : 128 * mi + 128],
                                xwm[ki][:, sl],
                                start=(ki == 0),
                                stop=(ki == 1),
                            )
                        if (2 * nt + mi + phase) % 2 == 0:
                            nc.scalar.activation(
                                ot[:, sl],
                                ps[:],
                                mybir.ActivationFunctionType.Identity,
                                bias=bias[bname][mi][:],
                            )
                        else:
                            nc.vector.tensor_scalar_add(
                                ot[:, sl], ps[:], bias[bname][mi][:]
                            )
                return out0, out1

            q = conv_qk("wqT", "bq", "q", 0)
            k = conv_qk("wkT", "bk", "k", 1)

            # ---- V pixel-major via transposed conv.  One matmul per
            # (pair, ki): stationary = 128-col slice of the 64-aligned
            # pair layout, so window a lands at PSUM partitions 0-48 and
            # window b at 64-112 (rows 49-63/113-127 are don't-care).
            vt = vtp.tile([C0, NW // 2, C], F16, tag="vt")
            for pp in range(0, NW // 2 if STAGE >= 2 else 0, 2):
                ps = psv.tile([C0, 2, C], F32, tag="v")
                prev_vstop = None
                for sub in range(2):
                    p = pp + sub
                    for ki in range(2):
                        mm = nc.tensor.matmul(
                            ps[:, sub],
                            xw2[ki][:, 128 * p : 128 * p + 128],
                            wt["wvT"][ki][:, 0:C],
                            start=(ki == 0), stop=(ki == 1),
                            skip_group_check=True,
                        )
                        if ki == 0 and prev_vstop is not None:
                            tile.add_dep_helper(
                                mm.ins, prev_vstop.ins, sync=True,
                                reason="v bank group order",
                            )
                        if ki == 1:
                            prev_vstop = mm
                nc.scalar.activation(
                    vt[0:WP, pp : pp + 2],
                    ps[0:WP],
                    mybir.ActivationFunctionType.Copy,
                )
                nc.vector.tensor_copy(
                    vt[64 : 64 + WP, pp : pp + 2], ps[64 : 64 + WP]
                )

            ot0 = otp.tile([C0, SP], F16, tag="ot0", bufs=3)
            ot1 = otp.tile([C0, SP], F16, tag="ot1", bufs=3)
            if STAGE >= 4:
                nc.gpsimd.memset(ot1[C1:C0], 0.0)  # K=128 padding rows
            if STAGE < 4:
                ot0, ot1 = q  # final conv consumes q; attention bypassed

            for g in range(NW // (2 * GP) if STAGE >= 3 else 0):
                # ---- S^T scores: 16 windows (8 pairs) -> one PSUM bank
                sc = pssc.tile([C0, GP, WP], F32, tag="sc")
                prev_stop = None
                for p in range(GP):
                    a = 2 * GP * g + 2 * p  # window index in strip
                    b = a + 1
                    # chain each group's start after the previous stop
                    for w, off, tp in ((a, 0, (0, 0)), (b, 64, (0, 64))):
                        for ki in range(2):
                            mm = nc.tensor.matmul(
                                sc[off : off + WP, p],
                                k[ki][:, WP * w : WP * w + WP],
                                q[ki][:, WP * w : WP * w + WP],
                                start=(ki == 0), stop=(ki == 1),
                                tile_position=tp,
                                skip_group_check=True,
                            )
                            if ki == 0 and prev_stop is not None:
                                tile.add_dep_helper(
                                    mm.ins, prev_stop.ins, sync=True,
                                    reason="qk bank group order",
                                )
                            if ki == 1:
                                prev_stop = mm
                # ---- exp(S^T/sqrt(C)) -> fp16 SBUF (valid slices only)
                expS = smp.tile([C0, GP, WP], F16, tag="exp", bufs=2)
                for off in (0, 64):
                    nc.scalar.activation(
                        expS[off : off + WP],
                        sc[off : off + WP],
                        mybir.ActivationFunctionType.Exp,
                        scale=SCALE,
                    )
                # ---- column sums via M=64 ones-stationaries -> pso bank
                cs = pso.tile([C0, GP, WP], F32, tag="podd")
                nc.tensor.matmul(
                    cs[0:64],
                    ones[0:WP, :],
                    expS[0:WP].rearrange("k p e -> k (p e)"),
                    start=True, stop=True,
                    tile_position=(0, 0),
                )
                nc.tensor.matmul(
                    cs[64:C0],
                    ones[64 : 64 + WP, :],
                    expS[64 : 64 + WP].rearrange("k p e -> k (p e)"),
                    start=True, stop=True,
                    tile_position=(64, 64),
                )
                rec = smp.tile([C0, GP, WP], F32, tag="rec", bufs=2)
                pT = smp.tile([C0, GP, WP], F16, tag="pT", bufs=2)
                nc.vector.reciprocal_approx_fast(rec[:], cs[:])
                for off in (0, 64):
                    nc.gpsimd.tensor_mul(
                        pT[off : off + WP],
                        expS[off : off + WP],
                        rec[off : off + WP],
                    )
                # ---- PV: O^T = V^T P^T.  m1 block first, then m0 block
                # (uniform tile configs within each block).  Even/odd out
                # partition ranges overlap -> separate banks.
                if STAGE < 4:
                    continue
                po1E = pso.tile([C1, GP, WP], F32, tag="podd")
                po1O = pso.tile([C1, GP, WP], F32, tag="podd")
                for p in range(GP):
                    vi = GP * g + p
                    nc.tensor.matmul(
                        po1E[:, p],
                        vt[0:WP, vi, C0:C],
                        pT[0:WP, p],
                        start=True, stop=True,
                        tile_position=(0, 0),
                    )
                    nc.tensor.matmul(
                        po1O[:, p],
                        vt[64 : 64 + WP, vi, C0:C],
                        pT[64 : 64 + WP, p],
                        start=True, stop=True,
                        tile_position=(64, 0),
                    )
                po0E = psb.tile([C0, GP, WP], F32, tag="big")
                po0O = psb.tile([C0, GP, WP], F32, tag="big")
                for p in range(GP):
                    vi = GP * g + p
                    nc.tensor.matmul(
                        po0E[:, p],
                        vt[0:WP, vi, 0:C0],
                        pT[0:WP, p],
                        start=True, stop=True,
                        tile_position=(0, 0),
                    )
                    nc.tensor.matmul(
                        po0O[:, p],
                        vt[64 : 64 + WP, vi, 0:C0],
                        pT[64 : 64 + WP, p],
                        start=True, stop=True,
                        tile_position=(64, 0),
                    )
                gsl = slice(2 * GP * WP * g, 2 * GP * WP * (g + 1))
                ot0v = ot0[:, gsl].rearrange(
                    "c (p par e) -> c par p e", par=2, e=WP
                )
                ot1v = ot1[:, gsl].rearrange(
                    "c (p par e) -> c par p e", par=2, e=WP
                )
                nc.scalar.activation(
                    ot0v[:, 0], po0E[:], mybir.ActivationFunctionType.Copy
                )
                nc.scalar.activation(
                    ot0v[:, 1], po0O[:], mybir.ActivationFunctionType.Copy
                )
                nc.vector.tensor_copy(ot1v[0:C1, 0], po1E[:])
                nc.vector.tensor_copy(ot1v[0:C1, 1], po1O[:])

            # ---- final conv + bias; evac converts window-major -> raster
            outs = (
                outp.tile([C0, WS, W], F32, tag="out0", name="out0t"),
                outp.tile([C1, WS, W], F32, tag="out1", name="out1t"),
            )
            for nt in range(NGRP):
                sl = slice(NT * nt, NT * nt + NT)
                for mi, msz in ((0, C0), (1, C1)):
                    ps = psb.tile([C0, NT], F32, tag="big")
                    for ki, ot in enumerate((ot0, ot1)):
                        nc.tensor.matmul(
                            ps[:],
                            wt["woT"][ki][:, 128 * mi : 128 * mi + 128],
                            ot[:, sl],
                            start=(ki == 0),
                            stop=(ki == 1),
                        )
                    ov = outs[mi][:].rearrange(
                        "c r (w cc) -> c w r cc", cc=WS
                    )[:, 8 * nt : 8 * nt + 8]
                    pv = ps[0:msz].rearrange(
                        "c (w r cc) -> c w r cc", r=WS, cc=WS
                    )
                    if (2 * nt + mi) % 2 == 0:
                        nc.scalar.activation(
                            ov, pv,
                            mybir.ActivationFunctionType.Identity,
                            bias=bias["bo"][mi][0:msz],
                        )
                    else:
                        nc.vector.tensor_scalar_add(
                            ov, pv, bias["bo"][mi][0:msz]
                        )
            nc.sync.dma_start(y_d[0:C0, 7 * s : 7 * s + 7, :], outs[0][:])
            nc.sync.dma_start(y_d[C0:C, 7 * s : 7 * s + 7, :], outs[1][:])

    nc.compile()
    return nc


def kernel(x, Wq, bq, Wk, bk, Wv, bv, Wo, bo):
    if "nc" not in _CACHE:
        _CACHE["nc"] = _build()
    nc = _CACHE["nc"]

    f32, f16 = np.float32, np.float16
    shared = {
        "wqT": np.ascontiguousarray(np.asarray(Wq, f32).T.astype(f16)),
        "wkT": np.ascontiguousarray(np.asarray(Wk, f32).T.astype(f16)),
        "wvT": np.ascontiguousarray(np.asarray(Wv, f32).T.astype(f16)),
        "woT": np.ascontiguousarray(np.asarray(Wo, f32).T.astype(f16)),
        "bq": np.ascontiguousarray(np.asarray(bq, f32).reshape(C, 1)),
        "bk": np.ascontiguousarray(np.asarray(bk, f32).reshape(C, 1)),
        "bo": np.ascontiguousarray(
            (np.asarray(Wo, f32) @ np.asarray(bv, f32) + np.asarray(bo, f32)).reshape(
                C, 1
            )
        ),
    }
    x = np.asarray(x, f32).astype(f16)
    # window-major layout: [C, strip, w, r, cc] flattened to [C, H*W]
    xw = np.ascontiguousarray(
        x.reshape(B, C, H // WS, WS, W // WS, WS)
        .transpose(0, 1, 2, 4, 3, 5)
        .reshape(B, C, H * W)
    )
    in_maps = [{"x": xw[b], **shared} for b in range(B)]
    res = run_bass_kernel_spmd(
        nc, in_maps, core_ids=list(range(B)), trace=TRACE
    )
    _CACHE["last_result"] = res
    return np.stack([r["y"] for r in res.results], axis=0)


TRACE = False


# revision 43
# speedup vs baseline: 1.7152x; 1.7152x over previous
"""LocalWindowAttention Trainium2 Bass kernel (v4: uniform-K fp16).

Full-input contract: kernel(**inputs) takes the unsharded tensors
(x:[8,192,224,224], Wq/Wk/Wv/Wo:[192,192], bq/bk/bv/bo:[192]) and
returns the full [8,192,224,224] output.  Data-parallel over batch
across 8 NeuronCores (1 image per core), weights replicated.

Math notes (vs reference):
  - H=W=224 divide by ws=7, so the reference's reflect-pad is a no-op.
  - V-bias folded out: softmax rows sum to 1, so bo_eff = Wo@bv + bo.
  - no max-subtraction in softmax: scores/sqrt(C) are O(+-6); exp fits
    fp16 range and fp32 PSUM accumulates the sums.
  - All matmuls fp16 (1 cyc/row), fp32 PSUM accumulation.

Performance-critical structure (measured on this part):
  - Back-to-back matmuls pipeline at ~N cycles ONLY when the tile
    config (rounded K x M) stays constant; alternating K=128/K=64
    costs ~2.7x.  So ALL channel-contraction matmuls are padded to
    K=128: the 64-row second channel-chunk of x/q/k and the weight
    chunks are zero-padded, and the q/k convs emit M=128 with 64 zero
    weight columns so the padded q1/k1 rows are zero by construction.
  - S^T formulation (lhsT=k, rhs=q) avoids P transposes entirely;
    window pairs use tile_position col groups 0/64 (PSUM partitions
    0-48 / 64-112, physically disjoint per-partition SRAMs).
  - V is computed pixel-major via a transposed conv (stationary =
    x-window chunk); PV uses V as stationary (moving = P^T),
    giving channel-major O^T.  Even/odd PV outputs overlap in
    partitions and therefore go to SEPARATE banks (concurrent
    row-tiled drains into one bank are a fatal HW collision).
  - Softmax: column sums via M=64 ones-stationary matmuls (sums
    replicated across partitions), fast approximate reciprocal,
    gpsimd multiply -> P^T fp16.
"""

import math
from contextlib import ExitStack

import numpy as np

import concourse.bacc as bacc
import concourse.bass as bass
import concourse.tile as tile
from concourse import mybir
from concourse.ap import AP
from concourse.bass_utils import run_bass_kernel_spmd

F32 = mybir.dt.float32
F16 = mybir.dt.float16

B, C, H, W = 8, 192, 224, 224
WS = 7
NSTRIP = H // WS            # 32 strips (one window-row each)
SP = WS * W                 # 1568 pixels per strip
NW = W // WS                # 32 windows per strip
WP = WS * WS                # 49 pixels per window
NT = 392                    # N-tile = 8 windows
NGRP = SP // NT             # 4 groups per strip
C0, C1 = 128, 64            # channel chunks (192 = 128 + 64)
SCALE = 1.0 / math.sqrt(C)
GP = 8                      # window pairs per attention group

_CACHE = {}

# debug: 1=convs only (final conv reads q), 2=+V, 3=+QK/softmax, 4=full
STAGE = 4


def _build():
    nc = bacc.Bacc(None, target_bir_lowering=False)

    x_d = nc.dram_tensor("x", [C, H * W], F16, kind="ExternalInput")
    xq_d = nc.dram_tensor("xq", [C, NSTRIP * 2048], F16, kind="ExternalInput")
    y_d = nc.dram_tensor("y", [C, H, W], F32, kind="ExternalOutput")
    w_d = {
        n: nc.dram_tensor(n, [C, C], F16, kind="ExternalInput")
        for n in ("wqT", "wkT", "wvT", "woT")
    }
    b_d = {
        n: nc.dram_tensor(n, [C, 1], F32, kind="ExternalInput")
        for n in ("bq", "bk", "bo")
    }
    ones_d = nc.inline_tensor(np.ones((C0, 64), dtype=np.float16), name="ones64")

    with tile.TileContext(nc) as tc, ExitStack() as ctx:
        const = ctx.enter_context(tc.tile_pool(name="const", bufs=1))

        # weights as two K=128 chunks of [128, 256]:
        #   wt[n][0]: rows = in-ch 0-127;  wt[n][1]: rows 0-63 = in-ch
        #   128-191, rows 64-127 = 0.  cols 0-191 = out-ch, 192-255 = 0
        #   (so the M=128 "m1" stationary wt[:,128:256] has 64 zero cols
        #   and the conv's second output chunk lands zero-padded).
        wt = {}
        for n in ("wqT", "wkT", "wvT", "woT"):
            t0 = const.tile([C0, 256], F16, tag=f"{n}0")
            t1 = const.tile([C0, 256], F16, tag=f"{n}1")
            nc.vector.memset(t0[:], 0.0)
            nc.vector.memset(t1[:], 0.0)
            nc.sync.dma_start(t0[:, 0:C], w_d[n][0:C0, :])
            nc.sync.dma_start(t1[0:C1, 0:C], w_d[n][C0:C, :])
            wt[n] = (t0, t1)
        bias = {}
        for n in ("bq", "bk", "bo"):
            t0 = const.tile([C0, 1], F32, tag=f"{n}0")
            t1 = const.tile([C0, 1], F32, tag=f"{n}1")
            nc.vector.memset(t1[:], 0.0)
            nc.sync.dma_start(t0[:], b_d[n][0:C0, :])
            nc.sync.dma_start(t1[0:C1], b_d[n][C0:C, :])
            bias[n] = (t0, t1)
        ones = const.tile([C0, 64], F16, tag="ones")
        nc.sync.dma_start(ones[:], ones_d[:, :])

        xp = ctx.enter_context(tc.tile_pool(name="xp", bufs=3))
        qkp = ctx.enter_context(tc.tile_pool(name="qkp", bufs=3))
        vtp = ctx.enter_context(tc.tile_pool(name="vtp", bufs=3))
        smp = ctx.enter_context(tc.tile_pool(name="smp", bufs=2))
        otp = ctx.enter_context(tc.tile_pool(name="otp", bufs=3))
        outp = ctx.enter_context(tc.tile_pool(name="outp", bufs=3))

        # PSUM pools: 8 banks.  Matmul outputs that overlap in partition
        # range must be in different banks (concurrent drains collide).
        psb = ctx.enter_context(
            tc.tile_pool(name="psb", bufs=3, space=bass.MemorySpace.PSUM)
        )  # [128,392] conv outs (m0+m1) and PV m0 even/odd
        pso = ctx.enter_context(
            tc.tile_pool(name="pso", bufs=2, space=bass.MemorySpace.PSUM)
        )  # colsums + PV m1 even/odd
        psv = ctx.enter_context(
            tc.tile_pool(name="psv", bufs=2, space=bass.MemorySpace.PSUM)
        )  # [128, 2, 192] V pairs (even at 0-48, odd at 64-112)
        pssc = ctx.enter_context(
            tc.tile_pool(name="pssc", bufs=1, space=bass.MemorySpace.PSUM)
        )  # [128, GP, 49] S^T

        for s in range(NSTRIP):
            # ---- x arrives from the host already window-major fp16
            # ([C, strip, w, r, cc] order): DMA straight into xwm.
            # xwm[1] rows 64-127 are zero (K=128 padding).
            xwm = (
                xp.tile([C0, SP], F16, tag="xw0", name="xw0t", bufs=3),
                xp.tile([C0, SP], F16, tag="xw1", name="xw1t", bufs=3),
            )
            nc.sync.dma_start(xwm[0][:], x_d[0:C0, SP * s : SP * (s + 1)])
            nc.sync.dma_start(
                xwm[1][0:C1], x_d[C0:C, SP * s : SP * (s + 1)]
            )
            nc.gpsimd.memset(xwm[1][C1:C0], 0.0)
            # 64-aligned window-pair layout for the V conv (from host):
            # pair p at cols 128p..+128, window a at +0, b at +64.
            xw2 = (
                xp.tile([C0, 2048], F16, tag="xq0", name="xq0t", bufs=3),
                xp.tile([C0, 2048], F16, tag="xq1", name="xq1t", bufs=3),
            )
            nc.sync.dma_start(
                xw2[0][:], xq_d[0:C0, 2048 * s : 2048 * (s + 1)]
            )
            nc.sync.dma_start(
                xw2[1][0:C1], xq_d[C0:C, 2048 * s : 2048 * (s + 1)]
            )
            nc.gpsimd.memset(xw2[1][C1:C0], 0.0)

            # ---- q, k convs -> two K=128-padded [128, 1568] chunks each
            # evacs alternate ACT/DVE per N-tile so neither engine gates
            def conv_qk(wname, bname, tag, phase):
                out0 = qkp.tile([C0, SP], F16, tag=f"{tag}0")
                out1 = qkp.tile([C0, SP], F16, tag=f"{tag}1")
                for nt in range(NGRP):
                    sl = slice(NT * nt, NT * nt + NT)
                    for mi, ot in ((0, out0), (1, out1)):
                        ps = psb.tile([C0, NT], F32, tag="big")
                        for ki in range(2):
                            nc.tensor.matmul(
                                ps[:],
                                wt[wname][ki][:, 128 * mi : 128 * mi + 128],
                                xwm[ki][:, sl],
                                start=(ki == 0),
                                stop=(ki == 1),
                            )
                        if (2 * nt + mi + phase) % 2 == 0:
                            nc.scalar.activation(
                                ot[:, sl],
                                ps[:],
                                mybir.ActivationFunctionType.Identity,
                                bias=bias[bname][mi][:],
                            )
                        else:
                            nc.vector.tensor_scalar_add(
                                ot[:, sl], ps[:], bias[bname][mi][:]
                            )
                return out0, out1

            q = conv_qk("wqT", "bq", "q", 0)
            k = conv_qk("wkT", "bk", "k", 1)

            # ---- V pixel-major via transposed conv.  One matmul per
            # (pair, ki): stationary = 128-col slice of the 64-aligned
            # pair layout, so window a lands at PSUM partitions 0-48 and
            # window b at 64-112 (rows 49-63/113-127 are don't-care).
            vt = vtp.tile([C0, NW // 2, C], F16, tag="vt")
            for pp in range(0, NW // 2 if STAGE >= 2 else 0, 2):
                ps = psv.tile([C0, 2, C], F32, tag="v")
                prev_vstop = None
                for sub in range(2):
                    p = pp + sub
                    for ki in range(2):
                        mm = nc.tensor.matmul(
                            ps[:, sub],
                            xw2[ki][:, 128 * p : 128 * p + 128],
                            wt["wvT"][ki][:, 0:C],
                            start=(ki == 0), stop=(ki == 1),
                            skip_group_check=True,
                        )
                        if ki == 0 and prev_vstop is not None:
                            tile.add_dep_helper(
                                mm.ins, prev_vstop.ins, sync=True,
                                reason="v bank group order",
                            )
                        if ki == 1:
                            prev_vstop = mm
                nc.scalar.activation(
                    vt[0:WP, pp : pp + 2],
                    ps[0:WP],
                    mybir.ActivationFunctionType.Copy,
                )
                nc.vector.tensor_copy(
                    vt[64 : 64 + WP, pp : pp + 2], ps[64 : 64 + WP]
                )

            ot0 = otp.tile([C0, SP], F16, tag="ot0", bufs=3)
            ot1 = otp.tile([C0, SP], F16, tag="ot1", bufs=3)
            if STAGE >= 4:
                nc.gpsimd.memset(ot1[C1:C0], 0.0)  # K=128 padding rows
            if STAGE < 4:
                ot0, ot1 = q  # final conv consumes q; attention bypassed

            for g in range(NW // (2 * GP) if STAGE >= 3 else 0):
                # ---- S^T scores: 16 windows (8 pairs) -> one PSUM bank
                sc = pssc.tile([C0, GP, WP], F32, tag="sc")
                prev_stop = None
                for p in range(GP):
                    a = 2 * GP * g + 2 * p  # window index in strip
                    b = a + 1
                    # chain each group's start after the previous stop
                    for w, off, tp in ((a, 0, (0, 0)), (b, 64, (0, 64))):
                        for ki in range(2):
                            mm = nc.tensor.matmul(
                                sc[off : off + WP, p],
                                k[ki][:, WP * w : WP * w + WP],
                                q[ki][:, WP * w : WP * w + WP],
                                start=(ki == 0), stop=(ki == 1),
                                tile_position=tp,
                                skip_group_check=True,
                            )
                            if ki == 0 and prev_stop is not None:
                                tile.add_dep_helper(
                                    mm.ins, prev_stop.ins, sync=True,
                                    reason="qk bank group order",
                                )
                            if ki == 1:
                                prev_stop = mm
                # ---- exp(S^T/sqrt(C)) -> fp16 SBUF (valid slices only)
                expS = smp.tile([C0, GP, WP], F16, tag="exp", bufs=2)
                for off in (0, 64):
                    nc.scalar.activation(
                        expS[off : off + WP],
                        sc[off : off + WP],
                        mybir.ActivationFunctionType.Exp,
                        scale=SCALE,
                    )
                # ---- column sums via M=64 ones-stationaries -> pso bank
                cs = pso.tile([C0, GP, WP], F32, tag="podd")
                nc.tensor.matmul(
                    cs[0:64],
                    ones[0:WP, :],
                    expS[0:WP].rearrange("k p e -> k (p e)"),
                    start=True, stop=True,
                    tile_position=(0, 0),
                )
                nc.tensor.matmul(
                    cs[64:C0],
                    ones[64 : 64 + WP, :],
                    expS[64 : 64 + WP].rearrange("k p e -> k (p e)"),
                    start=True, stop=True,
                    tile_position=(64, 64),
                )
                rec = smp.tile([C0, GP, WP], F32, tag="rec", bufs=2)
                pT = smp.tile([C0, GP, WP], F16, tag="pT", bufs=2)
                nc.vector.reciprocal_approx_fast(rec[:], cs[:])
                for off in (0, 64):
                    nc.gpsimd.tensor_mul(
                        pT[off : off + WP],
                        expS[off : off + WP],
                        rec[off : off + WP],
                    )
                # ---- PV: O^T = V^T P^T.  m1 block first, then m0 block
                # (uniform tile configs within each block).  Even/odd out
                # partition ranges overlap -> separate banks.
                if STAGE < 4:
                    continue
                po1E = pso.tile([C1, GP, WP], F32, tag="podd")
                po1O = pso.tile([C1, GP, WP], F32, tag="podd")
                for p in range(GP):
                    vi = GP * g + p
                    nc.tensor.matmul(
                        po1E[:, p],
                        vt[0:WP, vi, C0:C],
                        pT[0:WP, p],
                        start=True, stop=True,
                        tile_position=(0, 0),
                    )
                    nc.tensor.matmul(
                        po1O[:, p],
                        vt[64 : 64 + WP, vi, C0:C],
                        pT[64 : 64 + WP, p],
                        start=True, stop=True,
                        tile_position=(64, 0),
                    )
                po0E = psb.tile([C0, GP, WP], F32, tag="big")
                po0O = psb.tile([C0, GP, WP], F32, tag="big")
                for p in range(GP):
                    vi = GP * g + p
                    nc.tensor.matmul(
                        po0E[:, p],
                        vt[0:WP, vi, 0:C0],
                        pT[0:WP, p],
                        start=True, stop=True,
                        tile_position=(0, 0),
                    )
                    nc.tensor.matmul(
                        po0O[:, p],
                        vt[64 : 64 + WP, vi, 0:C0],
                        pT[64 : 64 + WP, p],
                        start=True, stop=True,
                        tile_position=(64, 0),
                    )
                gsl = slice(2 * GP * WP * g, 2 * GP * WP * (g + 1))
                ot0v = ot0[:, gsl].rearrange(
                    "c (p par e) -> c par p e", par=2, e=WP
                )
                ot1v = ot1[:, gsl].rearrange(
                    "c (p par e) -> c par p e", par=2, e=WP
                )
                nc.scalar.activation(
                    ot0v[:, 0], po0E[:], mybir.ActivationFunctionType.Copy
                )
                nc.scalar.activation(
                    ot0v[:, 1], po0O[:], mybir.ActivationFunctionType.Copy
                )
                nc.vector.tensor_copy(ot1v[0:C1, 0], po1E[:])
                nc.vector.tensor_copy(ot1v[0:C1, 1], po1O[:])

            # ---- final conv + bias; evac converts window-major -> raster
            outs = (
                outp.tile([C0, WS, W], F32, tag="out0", name="out0t"),
                outp.tile([C1, WS, W], F32, tag="out1", name="out1t"),
            )
            for nt in range(NGRP):
                sl = slice(NT * nt, NT * nt + NT)
                for mi, msz in ((0, C0), (1, C1)):
                    ps = psb.tile([C0, NT], F32, tag="big")
                    for ki, ot in enumerate((ot0, ot1)):
                        nc.tensor.matmul(
                            ps[:],
                            wt["woT"][ki][:, 128 * mi : 128 * mi + 128],
                            ot[:, sl],
                            start=(ki == 0),
                            stop=(ki == 1),
                        )
                    ov = outs[mi][:].rearrange(
                        "c r (w cc) -> c w r cc", cc=WS
                    )[:, 8 * nt : 8 * nt + 8]
                    pv = ps[0:msz].rearrange(
                        "c (w r cc) -> c w r cc", r=WS, cc=WS
                    )
                    if (2 * nt + mi) % 2 == 0:
                        nc.scalar.activation(
                            ov, pv,
                            mybir.ActivationFunctionType.Identity,
                            bias=bias["bo"][mi][0:msz],
                        )
                    else:
                        nc.vector.tensor_scalar_add(
                            ov, pv, bias["bo"][mi][0:msz]
                        )
            nc.sync.dma_start(y_d[0:C0, 7 * s : 7 * s + 7, :], outs[0][:])
            nc.sync.dma_start(y_d[C0:C, 7 * s : 7 * s + 7, :], outs[1][:])

    nc.compile()
    return nc


def kernel(x, Wq, bq, Wk, bk, Wv, bv, Wo, bo):
    if "nc" not in _CACHE:
        _CACHE["nc"] = _build()
    nc = _CACHE["nc"]

    f32, f16 = np.float32, np.float16
    shared = {
        "wqT": np.ascontiguousarray(np.asarray(Wq, f32).T.astype(f16)),
        "wkT": np.ascontiguousarray(np.asarray(Wk, f32).T.astype(f16)),
        "wvT": np.ascontiguousarray(np.asarray(Wv, f32).T.astype(f16)),
        "woT": np.ascontiguousarray(np.asarray(Wo, f32).T.astype(f16)),
        "bq": np.ascontiguousarray(np.asarray(bq, f32).reshape(C, 1)),
        "bk": np.ascontiguousarray(np.asarray(bk, f32).reshape(C, 1)),
        "bo": np.ascontiguousarray(
            (np.asarray(Wo, f32) @ np.asarray(bv, f32) + np.asarray(bo, f32)).reshape(
                C, 1
            )
        ),
    }
    x = np.asarray(x, f32).astype(f16)
    # window-major layout: [C, strip, w, r, cc] flattened to [C, H*W]
    xw = np.ascontiguousarray(
        x.reshape(B, C, H // WS, WS, W // WS, WS)
        .transpose(0, 1, 2, 4, 3, 5)
        .reshape(B, C, H * W)
    )
    xq = np.zeros((B, C, H // WS, W // WS // 2, 2, 64), f16)
    xq[..., 0:WP] = (
        x.reshape(B, C, H // WS, WS, W // WS, WS)
        .transpose(0, 1, 2, 4, 3, 5)
        .reshape(B, C, H // WS, W // WS // 2, 2, WP)
    )
    xq = np.ascontiguousarray(xq.reshape(B, C, NSTRIP * 2048))
    in_maps = [
        {"x": xw[b], "xq": xq[b], **shared} for b in range(B)
    ]
    res = run_bass_kernel_spmd(
        nc, in_maps, core_ids=list(range(B)), trace=TRACE
    )
    _CACHE["last_result"] = res
    return np.stack([r["y"] for r in res.results], axis=0)


TRACE = False
